# revision 1
# baseline (speedup 1.0000x reference)
"""CopyGenerator kernel for 8x Trainium2 NeuronCores (Bass/Tile).

Computation (see reference):
    logits = hidden @ W.T + b            [BT, V]   (pad column masked to -inf)
    prob   = softmax(logits, axis=1)
    p_copy = sigmoid(hidden @ w_copy + b_copy)
    out    = concat([prob * (1 - p_copy),
                     einsum('bts,bsc', attn*p_copy, src_map)], axis=1)

Sharding: vocab dim of W/b/out_prob split 8 ways (tensor parallel).  Each
core computes exp(logits) for its vocab shard (bf16 matmul, tokens on
PSUM partitions), a per-token local sum-of-exp, then an 8-core AllReduce
of the [BT] normalizer (tiny), and scales+writes its out_prob columns.
The copy branch is data-parallel over batch (2 batches per core).
"""

import sys

for _p in ("/opt/trn_rl_repo", "/root/.axon_site/_ro/trn_rl_repo"):
    if _p not in sys.path:
        sys.path.insert(0, _p)

import numpy as np

import concourse.bass as bass
import concourse.mybir as mybir
from concourse import bacc, tile
from concourse.bass_utils import run_bass_kernel_spmd
from concourse.masks import make_identity

f32 = mybir.dt.float32
bf16 = mybir.dt.bfloat16
P = 128

FULL_CFG = dict(B=16, T=128, S=512, C=512, V=50000, D=1024)
NCORES = 8


def _ceil_div(a, b):
    return (a + b - 1) // b


def build_nc(cfg):
    B, T, S, C, V, D = (cfg[k] for k in ("B", "T", "S", "C", "V", "D"))
    BT = B * T
    VSH = V // NCORES           # vocab columns per core
    NT = BT // P                # token tiles of 128
    NK = D // P                 # contraction k-tiles
    NVT = _ceil_div(VSH, 512)   # vocab tiles of <=512
    NS = S // P                 # copy-branch contraction k-tiles
    BSH = B // NCORES           # batches per core (copy branch)
    if NT > 6:
        body = NT - 4
        a = (body + 2) // 3
        b = (body - a + 1) // 2
        c = body - a - b
        GROUP_SIZES = [g for g in (a, b, c, 2, 1, 1) if g > 0]
    else:
        GROUP_SIZES = [NT - 1, 1] if NT > 1 else [NT]
    NG = len(GROUP_SIZES)
    OUTW = 512                  # out-staging width (columns per store DMA)

    nc = bacc.Bacc(
        "TRN2", target_bir_lowering=False, debug=False, num_devices=NCORES
    )
    hidden = nc.declare_dram_parameter("hidden", [BT, D], f32, isOutput=False)
    w_sh = nc.declare_dram_parameter("w_shard", [VSH, D], f32, isOutput=False)
    b_sh = nc.declare_dram_parameter("b_shard", [1, VSH], bf16, isOutput=False)
    wcp = nc.declare_dram_parameter("w_copyT", [P, NK], bf16, isOutput=False)
    bcp = nc.declare_dram_parameter("b_copy", [1, 1], bf16, isOutput=False)
    attn_sh = nc.declare_dram_parameter("attn_shard", [BSH * T, S], f32, isOutput=False)
    src_sh = nc.declare_dram_parameter("src_shard", [BSH, S, C], f32, isOutput=False)
    hid_cb = nc.declare_dram_parameter("hidden_cb", [BSH * T, D], f32, isOutput=False)
    out_p = nc.declare_dram_parameter("out_prob", [BT, VSH], f32, isOutput=True)
    out_c = nc.declare_dram_parameter("copy_prob", [BSH * T, C], f32, isOutput=True)

    Exp = mybir.ActivationFunctionType.Exp
    add = mybir.AluOpType.add
    mult = mybir.AluOpType.mult

    with tile.TileContext(nc, num_cores=NCORES) as tc:
        from contextlib import ExitStack

        with ExitStack() as stack:
            constp = stack.enter_context(tc.tile_pool(name="const", bufs=1))
            persist = stack.enter_context(tc.tile_pool(name="persist", bufs=1))
            wstgp = stack.enter_context(tc.tile_pool(name="wstgp", bufs=2))
            hstgp = stack.enter_context(tc.tile_pool(name="hstgp", bufs=2))
            htp = stack.enter_context(tc.tile_pool(name="hT", bufs=3))
            sumsp = stack.enter_context(tc.tile_pool(name="sums", bufs=3))
            outstp = stack.enter_context(tc.tile_pool(name="outst", bufs=2))
            smallp = stack.enter_context(tc.tile_pool(name="small", bufs=8))
            lsgp = stack.enter_context(tc.tile_pool(name="lsg", bufs=4))
            cbp = stack.enter_context(tc.tile_pool(name="cb", bufs=1))
            astgp = stack.enter_context(tc.tile_pool(name="astgp", bufs=1))
            srcp = stack.enter_context(tc.tile_pool(name="srcp", bufs=1))
            psmm = stack.enter_context(
                tc.tile_pool(name="psum_mm", bufs=4, space="PSUM"))
            pstr = stack.enter_context(
                tc.tile_pool(name="psum_tr", bufs=3, space="PSUM"))
            pssm = stack.enter_context(
                tc.tile_pool(name="psum_sm", bufs=1, space="PSUM"))
            dramp = stack.enter_context(
                tc.tile_pool(name="ccdram", bufs=2 * NG, space="DRAM"))
            scrp = stack.enter_context(
                tc.tile_pool(name="scrdram", bufs=1, space="DRAM"))

            # ---- constants ----
            ident_bf = constp.tile([P, P], bf16)
            make_identity(nc, ident_bf)
            ident_f = constp.tile([P, P], f32)
            make_identity(nc, ident_f)
            wcT = constp.tile([P, NK], bf16)
            nc.sync.dma_start(wcT[:, :], wcp.ap())
            ones1 = constp.tile([1, P], bf16)
            nc.gpsimd.memset(ones1[:, :], 1.0)
            bcT = constp.tile([1, 1], bf16)
            nc.sync.dma_start(bcT[:, :], bcp.ap())
            bc_ps = pssm.tile([P, 1], f32, tag="pc", bufs=1)
            nc.tensor.matmul(bc_ps[:, :], ones1[0:1, :], bcT[0:1, :],
                             start=True, stop=True)
            bcNeg = constp.tile([P, 1], f32)
            nc.vector.tensor_scalar(bcNeg[:, :], bc_ps[:, :], -1.0, None, mult)

            # ---- bias broadcast [P, VSH] bf16 (b_row pool scoped) ----
            b_bc = persist.tile([P, VSH], bf16)
            with tc.tile_pool(name="bload", bufs=1) as blp:
                b_row = blp.tile([1, VSH], bf16)
                nc.sync.dma_start(b_row[:, :], b_sh.ap())
                for vt in range(NVT):
                    c0 = vt * 512
                    nsz = min(512, VSH - c0)
                    pm = psmm.tile([P, 512], f32, tag="mm")
                    nc.tensor.matmul(
                        pm[:, :nsz], ones1[0:1, :], b_row[0:1, c0 : c0 + nsz],
                        start=True, stop=True,
                    )
                    nc.vector.tensor_copy(out=b_bc[:, c0 : c0 + nsz],
                                          in_=pm[:, :nsz])

            # exp staging + C-phase landing pools (opened after bload closes)
            expp = stack.enter_context(tc.tile_pool(name="exp", bufs=2))
            landp = stack.enter_context(tc.tile_pool(name="land", bufs=2))

            # DRAM scratch for unnormalized exp (bf16)
            exp_scr = scrp.tile([BT, VSH], bf16)

            # ---- W shard -> per-vt wT tiles [P(d), NK, nsz] bf16 ----
            wT_t = []
            for vt in range(NVT):
                nsz = min(512, VSH - vt * 512)
                wtile = persist.tile([P, NK, nsz], bf16, name=f"wT{vt}")
                wT_t.append(wtile)
                for ch in range(_ceil_div(nsz, P)):
                    r0 = vt * 512 + ch * P
                    rows = min(P, VSH - r0)
                    wstg = wstgp.tile([P, D], f32, tag="wstg")
                    nc.sync.dma_start(
                        wstg[:rows, :], w_sh.ap()[r0 : r0 + rows, :]
                    )
                    for k in range(NK):
                        ps = pstr.tile([P, P], f32, tag="trps_f")
                        nc.tensor.transpose(
                            ps[:, :rows],
                            wstg[:rows, k * P : (k + 1) * P],
                            ident_f[:rows, :rows],
                        )
                        nc.vector.tensor_copy(
                            out=wtile[:, k, ch * P : ch * P + rows],
                            in_=ps[:, :rows],
                        )

            # ---- copy branch (batch-parallel; independent of main loop) ----
            for i in range(BSH):
                hstg = hstgp.tile([P, D], f32, tag="hstg")
                nc.sync.dma_start(hstg[:, :], hid_cb.ap()[i * P : (i + 1) * P, :])
                hTc = htp.tile([P, NK, P], bf16, tag="hT")
                for k in range(NK):
                    ps = pstr.tile([P, P], f32, tag="trps_f")
                    nc.tensor.transpose(
                        ps[:, :], hstg[:, k * P : (k + 1) * P], ident_f[:, :]
                    )
                    nc.vector.tensor_copy(out=hTc[:, k, :], in_=ps[:, :])
                pps = pssm.tile([P, 1], f32, tag="pc", bufs=1)
                for k in range(NK):
                    nc.tensor.matmul(
                        pps[:, :], hTc[:, k, :], wcT[:, k : k + 1],
                        start=(k == 0), stop=(k == NK - 1),
                    )
                ycb = smallp.tile([P, 1], f32, tag="sc")
                nc.scalar.activation(
                    ycb[:, :], pps[:, :], Exp, bias=bcNeg[:, :], scale=-1.0,
                )
                t1 = smallp.tile([P, 1], f32, tag="sc")
                nc.vector.tensor_scalar(t1[:, :], ycb[:, :], 1.0, None, add)
                pcb = smallp.tile([P, 1], f32, tag="sc")
                nc.vector.reciprocal(pcb[:, :], t1[:, :])

                astg = astgp.tile([P, S], f32, tag="astg")
                nc.sync.dma_start(astg[:, :], attn_sh.ap()[i * P : (i + 1) * P, :])
                amul = cbp.tile([P, S], f32, tag="amul")
                nc.vector.tensor_scalar(amul[:, :], astg[:, :], pcb[:, :], None, mult)
                aT = cbp.tile([P, NS, P], f32, tag="aT")
                for k in range(NS):
                    ps = pstr.tile([P, P], f32, tag="trps_f")
                    nc.tensor.transpose(
                        ps[:, :], amul[:, k * P : (k + 1) * P], ident_f[:, :]
                    )
                    nc.vector.tensor_copy(out=aT[:, k, :], in_=ps[:, :])
                srcT = srcp.tile([P, NS, C], f32, tag="srcT")
                for k in range(NS):
                    nc.sync.dma_start(
                        srcT[:, k, :], src_sh.ap()[i, k * P : (k + 1) * P, :]
                    )
                cps = psmm.tile([P, C], f32, tag="mm")
                for k in range(NS):
                    nc.tensor.matmul(
                        cps[:, :], aT[:, k, :], srcT[:, k, :],
                        start=(k == 0), stop=(k == NS - 1),
                    )
                cstg = cbp.tile([P, C], f32, tag="cstg")
                nc.vector.tensor_copy(out=cstg[:, :], in_=cps[:, :])
                nc.sync.dma_start(out_c.ap()[i * P : (i + 1) * P, :], cstg[:, :])

            # ---- main loop ----
            pcall = persist.tile([P, NT], f32)
            S_all = persist.tile([P, NT], f32)

            def phase_a(tt):
                hstg = hstgp.tile([P, D], f32, tag="hstg")
                nc.sync.dma_start(hstg[:, :], hidden.ap()[tt * P : (tt + 1) * P, :])
                hT = htp.tile([P, NK, P], bf16, tag="hT")
                for k in range(NK):
                    ps = pstr.tile([P, P], f32, tag="trps_f")
                    nc.tensor.transpose(
                        ps[:, :], hstg[:, k * P : (k + 1) * P], ident_f[:, :]
                    )
                    nc.vector.tensor_copy(out=hT[:, k, :], in_=ps[:, :])
                pps = pssm.tile([P, 1], f32, tag="pc", bufs=1)
                for k in range(NK):
                    nc.tensor.matmul(
                        pps[:, :], hT[:, k, :], wcT[:, k : k + 1],
                        start=(k == 0), stop=(k == NK - 1),
                    )
                nc.scalar.activation(
                    pcall[:, tt : tt + 1], pps[:, :], Exp,
                    bias=bcNeg[:, :], scale=-1.0,
                )
                expstg = expp.tile([P, VSH], bf16, tag="exp")
                sums_vt = sumsp.tile([P, NVT], f32, tag="sums")
                for vt in range(NVT):
                    c0 = vt * 512
                    nsz = min(512, VSH - c0)
                    pm = psmm.tile([P, 512], f32, tag="mm")
                    for k in range(NK):
                        nc.tensor.matmul(
                            pm[:, :nsz], hT[:, k, :], wT_t[vt][:, k, :nsz],
                            start=(k == 0), stop=(k == NK - 1),
                        )
                    nc.vector.tensor_tensor(
                        pm[:, :nsz], pm[:, :nsz], b_bc[:, c0 : c0 + nsz], add
                    )
                    nc.scalar.activation(
                        expstg[:, c0 : c0 + nsz], pm[:, :nsz], Exp,
                        accum_out=sums_vt[:, vt : vt + 1],
                    )
                nc.sync.dma_start(
                    exp_scr[tt * P : (tt + 1) * P, :], expstg[:, :]
                )
                return sums_vt

            def phase_c(tt):
                y = pcall[:, tt : tt + 1]
                t1 = smallp.tile([P, 1], f32, tag="sc")
                nc.vector.tensor_scalar(t1[:, :], y, 1.0, None, add)
                t2 = smallp.tile([P, 1], f32, tag="sc")
                nc.vector.tensor_tensor(t2[:, :], t1[:, :], S_all[:, tt : tt + 1], mult)
                t3 = smallp.tile([P, 1], f32, tag="sc")
                nc.vector.reciprocal(t3[:, :], t2[:, :])
                rs = smallp.tile([P, 1], f32, tag="sc")
                nc.vector.tensor_tensor(rs[:, :], t3[:, :], y, mult)
                land = landp.tile([P, VSH], bf16, tag="land")
                nc.sync.dma_start(land[:, :], exp_scr[tt * P : (tt + 1) * P, :])
                for g0 in range(0, VSH, OUTW):
                    width = min(OUTW, VSH - g0)
                    outst = outstp.tile([P, OUTW], f32, tag="outst")
                    for c0 in range(g0, g0 + width, 512):
                        nsz = min(512, g0 + width - c0)
                        nc.vector.tensor_scalar(
                            outst[:, c0 - g0 : c0 - g0 + nsz],
                            land[:, c0 : c0 + nsz],
                            rs[:, :], None, mult,
                        )
                    nc.sync.dma_start(
                        out_p.ap()[tt * P : (tt + 1) * P, g0 : g0 + width],
                        outst[:, :width],
                    )

            groups = []
            tt0 = 0
            for gsz in GROUP_SIZES:
                groups.append(list(range(tt0, tt0 + gsz)))
                tt0 += gsz
            assert tt0 == NT

            # Emit C(g) one group AFTER A(g+1): by then the group-g
            # allreduce has completed, so C's DMAs never stall at the
            # head of the shared HWDGE FIFO and block A loads.
            for g, grp in enumerate(groups):
                lsg = lsgp.tile([P, len(grp)], f32, tag="lsg")
                for j, tt in enumerate(grp):
                    sums_vt = phase_a(tt)
                    nc.vector.tensor_reduce(
                        lsg[:, j : j + 1], sums_vt[:, :NVT],
                        mybir.AxisListType.X, add,
                    )
                cc_in = dramp.tile([P, len(grp)], f32, tag="cc_in")
                cc_out = dramp.tile([P, len(grp)], f32, tag="cc_out")
                nc.sync.dma_start(cc_in[:, :], lsg[:, :])
                nc.gpsimd.collective_compute(
                    "AllReduce", add,
                    replica_groups=[list(range(NCORES))],
                    ins=[cc_in.opt()], outs=[cc_out.opt()],
                )
                nc.sync.dma_start(
                    S_all[:, grp[0] : grp[0] + len(grp)], cc_out[:, :]
                )
                if g >= 1:
                    for tt in groups[g - 1]:
                        phase_c(tt)
            for tt in groups[-1]:
                phase_c(tt)

    nc.finalize()
    return nc


_CACHE = {}


def _get_nc(key, cfg):
    if key not in _CACHE:
        _CACHE[key] = build_nc(cfg)
    return _CACHE[key]


def make_in_maps(cfg, hidden, attn, src_map, W, b, w_copy, b_copy, pad_idx):
    B, T, S, C, V, D = (cfg[k] for k in ("B", "T", "S", "C", "V", "D"))
    BT = B * T
    VSH = V // NCORES
    BSH = B // NCORES
    hidden = np.ascontiguousarray(np.asarray(hidden, dtype=np.float32))
    attn = np.ascontiguousarray(np.asarray(attn, dtype=np.float32))
    src_map = np.ascontiguousarray(np.asarray(src_map, dtype=np.float32))
    W = np.ascontiguousarray(np.asarray(W, dtype=np.float32))
    b = np.asarray(b, dtype=np.float32)
    import ml_dtypes

    bF = ml_dtypes.bfloat16
    w_copyT = np.ascontiguousarray(
        np.asarray(w_copy, dtype=np.float32).reshape(-1, P).T.astype(bF)
    )
    b_copy = np.asarray(b_copy, dtype=np.float32).reshape(1, 1).astype(bF)
    pad = int(np.asarray(pad_idx))

    in_maps = []
    for c in range(NCORES):
        bsl = b[c * VSH : (c + 1) * VSH].copy()
        lo, hi = c * VSH, (c + 1) * VSH
        if lo <= pad < hi:
            bsl[pad - lo] = -1e30
        bsl = bsl.astype(bF)
        in_maps.append(
            {
                "hidden": hidden,
                "w_shard": np.ascontiguousarray(W[lo:hi]),
                "b_shard": np.ascontiguousarray(bsl.reshape(1, VSH)),
                "w_copyT": w_copyT,
                "b_copy": b_copy,
                "attn_shard": np.ascontiguousarray(
                    attn[c * BSH * T : (c + 1) * BSH * T]
                ),
                "src_shard": np.ascontiguousarray(src_map[c * BSH : (c + 1) * BSH]),
                "hidden_cb": np.ascontiguousarray(
                    hidden[c * BSH * T : (c + 1) * BSH * T]
                ),
            }
        )
    return in_maps


def assemble(cfg, results):
    out_prob = np.concatenate([r["out_prob"] for r in results], axis=1)
    copy_prob = np.concatenate([r["copy_prob"] for r in results], axis=0)
    return np.concatenate([out_prob, copy_prob], axis=1)


def run(cfg, inputs, trace=False):
    nc = _get_nc(tuple(sorted(cfg.items())), cfg)
    in_maps = make_in_maps(cfg, **inputs)
    res = run_bass_kernel_spmd(
        nc, in_maps, list(range(NCORES)), trace=trace
    )
    return assemble(cfg, res.results), res


def kernel(**inputs) -> np.ndarray:
    out, _ = run(FULL_CFG, inputs, trace=False)
    return out



# revision 4
# speedup vs baseline: 1.5596x; 1.5596x over previous
"""CopyGenerator kernel for 8x Trainium2 NeuronCores (Bass/Tile).

Computation (see reference):
    logits = hidden @ W.T + b            [BT, V]   (pad column masked to -inf)
    prob   = softmax(logits, axis=1)
    p_copy = sigmoid(hidden @ w_copy + b_copy)
    out    = concat([prob * (1 - p_copy),
                     einsum('bts,bsc', attn*p_copy, src_map)], axis=1)

Sharding: vocab dim of W/b/out_prob split 8 ways (tensor parallel).
All operand transposes are done on the host (free); the device only does
matmuls + exp + scaling.  W^T stays resident in SBUF (bf16), exp(logits)
stays in SBUF (never round-trips to DRAM), and the per-token normalizer
is AllReduced across cores in groups of 2 token tiles, pipelined behind
the next group's matmuls.  The copy branch is data-parallel over batch
(2 batches per core); p_copy is folded in after the einsum (linearity)
so no on-device transpose of attn is needed.
"""

import sys

for _p in ("/opt/trn_rl_repo", "/root/.axon_site/_ro/trn_rl_repo"):
    if _p not in sys.path:
        sys.path.insert(0, _p)

import numpy as np

import concourse.bass as bass
import concourse.mybir as mybir
from concourse import bacc, tile
from concourse.bass_utils import run_bass_kernel_spmd

f32 = mybir.dt.float32
bf16 = mybir.dt.bfloat16
P = 128

B, T, S, C, V, D = 16, 128, 512, 512, 50000, 1024
BT = B * T
NCORES = 8
VSH = V // NCORES            # 6250 vocab columns per core
VSHP = 6272                  # padded to 49*128 (pad cols get b=-1e30 -> exp=0)
NK = D // P                  # 8 contraction k-tiles
NT = BT // P                 # 16 token tiles
NS = S // P                  # 4 copy-branch contraction k-tiles
BSH = B // NCORES            # 2 batches per core (copy branch)
GROUP = 2                    # token tiles per normalizer AllReduce
NG = NT // GROUP
VG_BANKS = 6                 # PSUM banks per vocab sweep group
VGW = VG_BANKS * 512
NVT = (VSHP + 511) // 512    # 13 vocab slices of <=512


def build_nc():
    nc = bacc.Bacc(
        "TRN2", target_bir_lowering=False, debug=False, num_devices=NCORES
    )
    # [tt][din][k*128+t] = hidden[tt*128+t, k*128+din]
    hT_d = nc.declare_dram_parameter("hT", [NT, P, D], bf16, isOutput=False)
    # [k][din][v] = W_shard[v, k*128+din]
    wT_d = nc.declare_dram_parameter("wT", [NK, P, VSHP], bf16, isOutput=False)
    b_d = nc.declare_dram_parameter("b_row", [1, VSHP], bf16, isOutput=False)
    wc_d = nc.declare_dram_parameter("w_copyT", [P, NK], bf16, isOutput=False)
    bc_d = nc.declare_dram_parameter("b_copy", [1, 1], bf16, isOutput=False)
    # [ks][s][t] = attn_shard[t, ks*128+s]
    at_d = nc.declare_dram_parameter("attnT", [NS, P, BSH * T], bf16, isOutput=False)
    # [i*NS+ks][s][c] = src_map[i, ks*128+s, c]
    src_d = nc.declare_dram_parameter("srcm", [BSH * NS, P, C], bf16, isOutput=False)
    hcb_d = nc.declare_dram_parameter("hidden_cb", [BSH, P, D], bf16, isOutput=False)
    out_p = nc.declare_dram_parameter("out_prob", [BT, VSHP], f32, isOutput=True)
    out_c = nc.declare_dram_parameter("copy_prob", [BSH * T, C], f32, isOutput=True)

    Exp = mybir.ActivationFunctionType.Exp
    add = mybir.AluOpType.add
    mult = mybir.AluOpType.mult

    with tile.TileContext(nc, num_cores=NCORES) as tc:
        from contextlib import ExitStack

        with ExitStack() as stack:
            constp = stack.enter_context(tc.tile_pool(name="const", bufs=1))
            wpool = stack.enter_context(tc.tile_pool(name="wres", bufs=1))
            htp = stack.enter_context(tc.tile_pool(name="hT", bufs=4))
            sumsp = stack.enter_context(tc.tile_pool(name="sums", bufs=3))
            smallp = stack.enter_context(tc.tile_pool(name="small", bufs=8))
            lsgp = stack.enter_context(tc.tile_pool(name="lsg", bufs=4))
            psmm = stack.enter_context(
                tc.tile_pool(name="psum_mm", bufs=VG_BANKS, space="PSUM"))
            pssm = stack.enter_context(
                tc.tile_pool(name="psum_sm", bufs=1, space="PSUM"))
            pscb = stack.enter_context(
                tc.tile_pool(name="psum_cb", bufs=1, space="PSUM"))
            dramp = stack.enter_context(
                tc.tile_pool(name="ccdram", bufs=2 * NG, space="DRAM"))

            # ---- tiny constants ----
            ones1 = constp.tile([1, P], bf16)
            nc.gpsimd.memset(ones1[:, :], 1.0)
            wcT = constp.tile([P, NK], bf16)
            nc.sync.dma_start(wcT[:, :], wc_d.ap())
            bcT = constp.tile([1, 1], bf16)
            nc.sync.dma_start(bcT[:, :], bc_d.ap())
            b_row = constp.tile([1, VSHP], bf16)
            nc.sync.dma_start(b_row[:, :], b_d.ap())
            bc_ps = pssm.tile([P, 1], f32, tag="pc", bufs=1)
            nc.tensor.matmul(bc_ps[:, :], ones1[0:1, :], bcT[0:1, :],
                             start=True, stop=True)
            bcNeg = constp.tile([P, 1], f32)
            nc.vector.tensor_scalar(bcNeg[:, :], bc_ps[:, :], -1.0, None, mult)

            pcall = constp.tile([P, NT], f32)
            S_all = constp.tile([P, NT], f32)

            # ---- prefetch first hidden tiles, then stream W (biggest) ----
            hT_t = {}
            for tt in range(min(4, NT)):
                t_ = htp.tile([P, D], bf16, tag="hT")
                nc.sync.dma_start(t_[:, :], hT_d.ap()[tt])
                hT_t[tt] = t_
            w_t = []
            for k in range(NK):
                wt = wpool.tile([P, VSHP], bf16, name=f"w{k}")
                nc.sync.dma_start(wt[:, :], wT_d.ap()[k])
                w_t.append(wt)

            # ---- bias broadcast b_bc[P, VSHP] (pad cols already -1e30) ----
            b_bc = wpool.tile([P, VSHP], bf16, name="b_bc")
            for vt in range(NVT):
                c0 = vt * 512
                w = min(512, VSHP - c0)
                pm = psmm.tile([P, 512], f32, tag="mm")
                nc.tensor.matmul(pm[:, :w], ones1[0:1, :], b_row[0:1, c0:c0 + w],
                                 start=True, stop=True)
                nc.vector.tensor_copy(out=b_bc[:, c0:c0 + w], in_=pm[:, :w])

            # ---- copy branch (batch-parallel, off the critical path) ----
            with tc.tile_pool(name="cb", bufs=1) as cbp:
                attnT = cbp.tile([P, NS, BSH * T], bf16)
                for ks in range(NS):
                    nc.sync.dma_start(attnT[:, ks, :], at_d.ap()[ks])
                for i in range(BSH):
                    hcb = cbp.tile([P, D], bf16, name=f"hcb{i}")
                    nc.sync.dma_start(hcb[:, :], hcb_d.ap()[i])
                    srcT = cbp.tile([P, NS, C], bf16, name=f"src{i}")
                    for ks in range(NS):
                        nc.sync.dma_start(srcT[:, ks, :], src_d.ap()[i * NS + ks])
                    pps = pssm.tile([P, 1], f32, tag="pc", bufs=1)
                    for k in range(NK):
                        nc.tensor.matmul(
                            pps[:, :], hcb[:, k * P:(k + 1) * P], wcT[:, k:k + 1],
                            start=(k == 0), stop=(k == NK - 1))
                    ycb = smallp.tile([P, 1], f32, tag="sc")
                    nc.scalar.activation(ycb[:, :], pps[:, :], Exp,
                                         bias=bcNeg[:, :], scale=-1.0)
                    t1 = smallp.tile([P, 1], f32, tag="sc")
                    nc.vector.tensor_scalar(t1[:, :], ycb[:, :], 1.0, None, add)
                    pcb = smallp.tile([P, 1], f32, tag="sc")
                    nc.vector.reciprocal(pcb[:, :], t1[:, :])
                    cps = pscb.tile([P, C], f32, tag="cb", bufs=1)
                    for ks in range(NS):
                        nc.tensor.matmul(
                            cps[:, :], attnT[:, ks, i * P:(i + 1) * P],
                            srcT[:, ks, :],
                            start=(ks == 0), stop=(ks == NS - 1))
                    cstg = cbp.tile([P, C], f32, name=f"cst{i}")
                    nc.vector.tensor_scalar(cstg[:, :], cps[:, :], pcb[:, :],
                                            None, mult)
                    nc.sync.dma_start(out_c.ap()[i * P:(i + 1) * P, :],
                                      cstg[:, :])

            # exp stays in SBUF: 4 token tiles in flight (2 groups)
            expp = stack.enter_context(tc.tile_pool(name="exp", bufs=4))
            outsp = stack.enter_context(tc.tile_pool(name="outst", bufs=3))

            # ---- main loop ----
            def phase_a(tt, lsg, j):
                hT = hT_t.pop(tt)
                exp_t = expp.tile([P, VSHP], bf16, tag="exp")
                sums = sumsp.tile([P, NVT], f32, tag="sums")
                pps = pssm.tile([P, 1], f32, tag="pc", bufs=1)
                vt = 0
                for g0 in range(0, VSHP, VGW):
                    gw = min(VGW, VSHP - g0)
                    slices = []
                    for c0 in range(g0, g0 + gw, 512):
                        w = min(512, g0 + gw - c0)
                        pm = psmm.tile([P, 512], f32, tag="mm")
                        slices.append((c0, w, pm))
                    # hold hT[k] stationary across all banks of this sweep
                    for k in range(NK):
                        lhsT = hT[:, k * P:(k + 1) * P]
                        for (c0, w, pm) in slices:
                            nc.tensor.matmul(
                                pm[:, :w], lhsT, w_t[k][:, c0:c0 + w],
                                start=(k == 0), stop=(k == NK - 1))
                        if g0 == 0:
                            nc.tensor.matmul(
                                pps[:, :], lhsT, wcT[:, k:k + 1],
                                start=(k == 0), stop=(k == NK - 1))
                    for (c0, w, pm) in slices:
                        nc.vector.tensor_tensor(
                            pm[:, :w], pm[:, :w], b_bc[:, c0:c0 + w], add)
                        nc.scalar.activation(
                            exp_t[:, c0:c0 + w], pm[:, :w], Exp,
                            accum_out=sums[:, vt:vt + 1])
                        vt += 1
                nc.scalar.activation(pcall[:, tt:tt + 1], pps[:, :], Exp,
                                     bias=bcNeg[:, :], scale=-1.0)
                nc.vector.tensor_reduce(lsg[:, j:j + 1], sums[:, :NVT],
                                        mybir.AxisListType.X, add)
                nxt = tt + 4
                if nxt < NT:
                    t_ = htp.tile([P, D], bf16, tag="hT")
                    nc.sync.dma_start(t_[:, :], hT_d.ap()[nxt])
                    hT_t[nxt] = t_
                return exp_t

            def phase_c(tt, exp_t):
                y = pcall[:, tt:tt + 1]
                t1 = smallp.tile([P, 1], f32, tag="sc")
                nc.vector.tensor_scalar(t1[:, :], y, 1.0, None, add)
                t2 = smallp.tile([P, 1], f32, tag="sc")
                nc.vector.tensor_tensor(t2[:, :], t1[:, :],
                                        S_all[:, tt:tt + 1], mult)
                t3 = smallp.tile([P, 1], f32, tag="sc")
                nc.vector.reciprocal(t3[:, :], t2[:, :])
                rs = smallp.tile([P, 1], f32, tag="sc")
                nc.vector.tensor_tensor(rs[:, :], t3[:, :], y, mult)
                for c0 in range(0, VSHP, 2048):
                    w = min(2048, VSHP - c0)
                    ost = outsp.tile([P, 2048], f32, tag="outst")
                    nc.vector.tensor_scalar(ost[:, :w], exp_t[:, c0:c0 + w],
                                            rs[:, :], None, mult)
                    nc.sync.dma_start(
                        out_p.ap()[tt * P:(tt + 1) * P, c0:c0 + w],
                        ost[:, :w])

            exp_tiles = {}
            for g in range(NG):
                lsg = lsgp.tile([P, GROUP], f32, tag="lsg")
                for j in range(GROUP):
                    tt = g * GROUP + j
                    exp_tiles[tt] = phase_a(tt, lsg, j)
                cc_in = dramp.tile([P, GROUP], f32, tag="cc_in")
                cc_out = dramp.tile([P, GROUP], f32, tag="cc_out")
                nc.sync.dma_start(cc_in[:, :], lsg[:, :])
                nc.gpsimd.collective_compute(
                    "AllReduce", add,
                    replica_groups=[list(range(NCORES))],
                    ins=[cc_in.opt()], outs=[cc_out.opt()],
                )
                nc.sync.dma_start(
                    S_all[:, g * GROUP:(g + 1) * GROUP], cc_out[:, :])
                if g >= 1:
                    for tt in range((g - 1) * GROUP, g * GROUP):
                        phase_c(tt, exp_tiles.pop(tt))
            for tt in range((NG - 1) * GROUP, NT):
                phase_c(tt, exp_tiles.pop(tt))

    nc.finalize()
    return nc


_CACHE = {}


def _get_nc():
    if "nc" not in _CACHE:
        _CACHE["nc"] = build_nc()
    return _CACHE["nc"]


def make_in_maps(hidden, attn, src_map, W, b, w_copy, b_copy, pad_idx):
    import ml_dtypes

    bF = ml_dtypes.bfloat16
    hidden = np.asarray(hidden, np.float32)
    attn = np.asarray(attn, np.float32)
    src_map = np.asarray(src_map, np.float32)
    W = np.asarray(W, np.float32)
    b = np.asarray(b, np.float32)
    w_copy = np.asarray(w_copy, np.float32)
    b_copy = np.asarray(b_copy, np.float32)
    pad = int(np.asarray(pad_idx))

    # hidden^T tiles: [tt, din, k*128+t]
    H3 = hidden.reshape(NT, P, NK, P).transpose(0, 3, 2, 1)
    H3 = np.ascontiguousarray(H3.reshape(NT, P, D).astype(bF))
    wcT = np.ascontiguousarray(w_copy.reshape(NK, P).T.astype(bF))
    bc = np.ascontiguousarray(b_copy.reshape(1, 1).astype(bF))

    in_maps = []
    for c in range(NCORES):
        lo, hi = c * VSH, (c + 1) * VSH
        Wp = np.zeros((VSHP, D), np.float32)
        Wp[:VSH] = W[lo:hi]
        wT = np.ascontiguousarray(
            Wp.reshape(VSHP, NK, P).transpose(1, 2, 0).astype(bF))
        bsl = np.full((VSHP,), -1e30, np.float32)
        bsl[:VSH] = b[lo:hi]
        if lo <= pad < hi:
            bsl[pad - lo] = -1e30
        b_rowA = np.ascontiguousarray(bsl.reshape(1, VSHP).astype(bF))
        a_sl = attn[c * BSH * T:(c + 1) * BSH * T]
        attnT = np.ascontiguousarray(
            a_sl.reshape(BSH * T, NS, P).transpose(1, 2, 0).astype(bF))
        s_sl = src_map[c * BSH:(c + 1) * BSH]
        srcm = np.ascontiguousarray(
            s_sl.reshape(BSH * NS, P, C).astype(bF))
        in_maps.append({
            "hT": H3,
            "wT": wT,
            "b_row": b_rowA,
            "w_copyT": wcT,
            "b_copy": bc,
            "attnT": attnT,
            "srcm": srcm,
            "hidden_cb": np.ascontiguousarray(H3[c * BSH:(c + 1) * BSH]),
        })
    return in_maps


def assemble(results):
    out_prob = np.concatenate(
        [np.asarray(r["out_prob"], np.float32)[:, :VSH] for r in results],
        axis=1)
    copy_prob = np.concatenate(
        [np.asarray(r["copy_prob"], np.float32) for r in results], axis=0)
    return np.concatenate([out_prob, copy_prob], axis=1)


FULL_CFG = dict(B=B, T=T, S=S, C=C, V=V, D=D)


def run(cfg, inputs, trace=False):
    """test.py interface: run(K.FULL_CFG, np_inputs, trace=True)."""
    nc = _get_nc()
    in_maps = make_in_maps(**inputs)
    res = run_bass_kernel_spmd(nc, in_maps, list(range(NCORES)), trace=trace)
    return assemble(res.results), res


def kernel(**inputs) -> np.ndarray:
    out, _ = run(FULL_CFG, inputs, trace=False)
    return out


# revision 11
# speedup vs baseline: 1.5707x; 1.0071x over previous
"""CopyGenerator kernel for 8x Trainium2 NeuronCores (Bass/Tile).

Computation (see reference):
    logits = hidden @ W.T + b            [BT, V]   (pad column masked to -inf)
    prob   = softmax(logits, axis=1)
    p_copy = sigmoid(hidden @ w_copy + b_copy)
    out    = concat([prob * (1 - p_copy),
                     einsum('bts,bsc', attn*p_copy, src_map)], axis=1)

Sharding: vocab dim of W/b/out_prob split 8 ways (tensor parallel).
All operand transposes are done on the host (free); the device only does
matmuls + exp + scaling.  W^T stays resident in SBUF (bf16), exp(logits)
stays in SBUF (never round-trips to DRAM), and the per-token normalizer
is AllReduced across cores in groups of token tiles, pipelined behind
the next group's matmuls.  Matmuls use 1024-wide moving operands into
double-bank PSUM tiles to amortize per-instruction overhead.  The last
two groups are single tiles so the final (latency-bound, ~35us)
AllReduce has minimal work behind it; the copy branch is emitted at the
end to fill that AllReduce's shadow.
"""

import sys

for _p in ("/opt/trn_rl_repo", "/root/.axon_site/_ro/trn_rl_repo"):
    if _p not in sys.path:
        sys.path.insert(0, _p)

import numpy as np

import concourse.bass as bass
import concourse.mybir as mybir
from concourse import bacc, tile
from concourse.bass_utils import run_bass_kernel_spmd

f32 = mybir.dt.float32
bf16 = mybir.dt.bfloat16
P = 128

B, T, S, C, V, D = 16, 128, 512, 512, 50000, 1024
BT = B * T
NCORES = 8
VSH = V // NCORES            # 6250 vocab columns per core
VSHP = 6272                  # padded to 49*128 (pad cols get b=-1e30 -> exp=0)
NK = D // P                  # 8 contraction k-tiles
NT = BT // P                 # 16 token tiles
NS = S // P                  # 4 copy-branch contraction k-tiles
BSH = B // NCORES            # 2 batches per core (copy branch)
GROUPS = [2, 2, 2, 2, 2, 2, 2, 1, 1]   # token tiles per normalizer AllReduce
WSPL = 3072                  # W column-split point (per-k DMA granularity)
# vocab sweep: PSUM groups of 6x512-wide tiles (6 banks), then the tail
VGS = [(0, [512] * 6),
       (3072, [512] * 6),
       (6144, [128])]
NVT = sum(len(s) for _, s in VGS)     # 13 accum columns


def build_nc():
    nc = bacc.Bacc(
        "TRN2", target_bir_lowering=False, debug=False, num_devices=NCORES
    )
    # [tt][din][k*128+t] = hidden[tt*128+t, k*128+din]
    hT_d = nc.declare_dram_parameter("hT", [NT, P, D], bf16, isOutput=False)
    # [k][din][v] = W_shard[v, k*128+din]
    wT_d = nc.declare_dram_parameter("wT", [NK, P, VSHP], bf16, isOutput=False)
    b_d = nc.declare_dram_parameter("b_row", [1, VSHP], bf16, isOutput=False)
    wc_d = nc.declare_dram_parameter("w_copyT", [P, NK], bf16, isOutput=False)
    bc_d = nc.declare_dram_parameter("b_copy", [1, 1], bf16, isOutput=False)
    # [ks][s][t] = attn_shard[t, ks*128+s]
    at_d = nc.declare_dram_parameter("attnT", [NS, P, BSH * T], bf16, isOutput=False)
    # [i*NS+ks][s][c] = src_map[i, ks*128+s, c]
    src_d = nc.declare_dram_parameter("srcm", [BSH * NS, P, C], bf16, isOutput=False)
    hcb_d = nc.declare_dram_parameter("hidden_cb", [BSH, P, D], bf16, isOutput=False)
    out_p = nc.declare_dram_parameter("out_prob", [BT, VSHP], f32, isOutput=True)
    out_c = nc.declare_dram_parameter("copy_prob", [BSH * T, C], f32, isOutput=True)

    Exp = mybir.ActivationFunctionType.Exp
    add = mybir.AluOpType.add
    mult = mybir.AluOpType.mult

    with tile.TileContext(nc, num_cores=NCORES) as tc:
        from contextlib import ExitStack

        with ExitStack() as stack:
            constp = stack.enter_context(tc.tile_pool(name="const", bufs=1))
            wpool = stack.enter_context(tc.tile_pool(name="wres", bufs=1))
            htp = stack.enter_context(tc.tile_pool(name="hT", bufs=3))
            sumsp = stack.enter_context(tc.tile_pool(name="sums", bufs=3))
            smallp = stack.enter_context(tc.tile_pool(name="small", bufs=8))
            lsgp = stack.enter_context(tc.tile_pool(name="lsg", bufs=4))
            cbp = stack.enter_context(tc.tile_pool(name="cb", bufs=1))
            psmm = stack.enter_context(
                tc.tile_pool(name="psum_mm", bufs=6, space="PSUM"))
            pssm = stack.enter_context(
                tc.tile_pool(name="psum_sm", bufs=1, space="PSUM"))
            pscb = stack.enter_context(
                tc.tile_pool(name="psum_cb", bufs=1, space="PSUM"))
            dramp = stack.enter_context(
                tc.tile_pool(name="ccdram", bufs=2 * len(GROUPS), space="DRAM"))

            # ---- tiny constants ----
            ones1 = constp.tile([1, P], bf16)
            nc.gpsimd.memset(ones1[:, :], 1.0)
            wcT = constp.tile([P, NK], bf16)
            nc.sync.dma_start(wcT[:, :], wc_d.ap())
            bcT = constp.tile([1, 1], bf16)
            nc.sync.dma_start(bcT[:, :], bc_d.ap())
            bc_ps = pssm.tile([P, 1], f32, tag="pc", bufs=1)
            nc.tensor.matmul(bc_ps[:, :], ones1[0:1, :], bcT[0:1, :],
                             start=True, stop=True)
            bcNeg = constp.tile([P, 1], f32)
            nc.vector.tensor_scalar(bcNeg[:, :], bc_ps[:, :], -1.0, None, mult)

            pcall = constp.tile([P, NT], f32)
            S_all = constp.tile([P, NT], f32)

            # ---- copy-branch input DMAs issued early; compute at the end
            attnT = cbp.tile([P, NS, BSH * T], bf16)
            for ks in range(NS):
                nc.sync.dma_start(attnT[:, ks, :], at_d.ap()[ks])
            hcb_t, src_t = [], []
            for i in range(BSH):
                hcb = cbp.tile([P, D], bf16, name=f"hcb{i}")
                nc.sync.dma_start(hcb[:, :], hcb_d.ap()[i])
                hcb_t.append(hcb)
                srcT = cbp.tile([P, NS, C], bf16, name=f"src{i}")
                for ks in range(NS):
                    nc.sync.dma_start(srcT[:, ks, :], src_d.ap()[i * NS + ks])
                src_t.append(srcT)

            # ---- prefetch first hidden tiles, then stream W (biggest) ----
            hT_t = {}
            for tt in range(min(3, NT)):
                t_ = htp.tile([P, D], bf16, tag="hT")
                nc.sync.dma_start(t_[:, :], hT_d.ap()[tt])
                hT_t[tt] = t_
            # W split in column halves per k so the first sweep group's
            # matmuls only wait on the first 6.3MB of W, not all 12.6MB.
            w_a, w_b = [], []
            for k in range(NK):
                wa = wpool.tile([P, WSPL], bf16, name=f"wa{k}")
                nc.sync.dma_start(wa[:, :], wT_d.ap()[k, :, 0:WSPL])
                w_a.append(wa)
            for k in range(NK):
                wb = wpool.tile([P, VSHP - WSPL], bf16, name=f"wb{k}")
                nc.sync.dma_start(wb[:, :], wT_d.ap()[k, :, WSPL:VSHP])
                w_b.append(wb)

            def w_slice(k, c0, w):
                if c0 + w <= WSPL:
                    return w_a[k][:, c0:c0 + w]
                assert c0 >= WSPL
                return w_b[k][:, c0 - WSPL:c0 - WSPL + w]

            # ---- bias broadcast b_bc[P, VSHP] (pad cols already -1e30) ----
            b_bc = wpool.tile([P, VSHP], bf16, name="b_bc")
            with tc.tile_pool(name="brow", bufs=1) as browp:
                b_row = browp.tile([1, VSHP], bf16)
                nc.sync.dma_start(b_row[:, :], b_d.ap())
                for c0 in range(0, VSHP, 512):
                    w = min(512, VSHP - c0)
                    pm = psmm.tile([P, 512], f32, tag="mm")
                    nc.tensor.matmul(pm[:, :w], ones1[0:1, :],
                                     b_row[0:1, c0:c0 + w],
                                     start=True, stop=True)
                    nc.vector.tensor_copy(out=b_bc[:, c0:c0 + w], in_=pm[:, :w])

            # exp stays in SBUF: up to 4 token tiles in flight
            expp = stack.enter_context(tc.tile_pool(name="exp", bufs=4))
            outsp = stack.enter_context(tc.tile_pool(name="outst", bufs=2))

            # ---- main loop ----
            def phase_a(tt, lsg, j):
                hT = hT_t.pop(tt)
                exp_t = expp.tile([P, VSHP], bf16, tag="exp")
                sums = sumsp.tile([P, NVT], f32, tag="sums")
                pps = pssm.tile([P, 1], f32, tag="pc", bufs=1)
                vt = 0
                for gi, (g0, widths) in enumerate(VGS):
                    slices = []
                    c0 = g0
                    for w in widths:
                        pm = psmm.tile([P, 512], f32, tag="mm")
                        slices.append((c0, w, pm))
                        c0 += w
                    # hold hT[k] stationary across all banks of this sweep
                    for k in range(NK):
                        lhsT = hT[:, k * P:(k + 1) * P]
                        for (c0, w, pm) in slices:
                            nc.tensor.matmul(
                                pm[:, :w], lhsT, w_slice(k, c0, w),
                                start=(k == 0), stop=(k == NK - 1))
                        if gi == 0:
                            nc.tensor.matmul(
                                pps[:, :], lhsT, wcT[:, k:k + 1],
                                start=(k == 0), stop=(k == NK - 1))
                    for (c0, w, pm) in slices:
                        nc.vector.tensor_tensor(
                            pm[:, :w], pm[:, :w], b_bc[:, c0:c0 + w], add)
                        nc.scalar.activation(
                            exp_t[:, c0:c0 + w], pm[:, :w], Exp,
                            accum_out=sums[:, vt:vt + 1])
                        vt += 1
                nc.scalar.activation(pcall[:, tt:tt + 1], pps[:, :], Exp,
                                     bias=bcNeg[:, :], scale=-1.0)
                nc.vector.tensor_reduce(lsg[:, j:j + 1], sums[:, :NVT],
                                        mybir.AxisListType.X, add)
                nxt = tt + 3
                if nxt < NT:
                    t_ = htp.tile([P, D], bf16, tag="hT")
                    nc.sync.dma_start(t_[:, :], hT_d.ap()[nxt])
                    hT_t[nxt] = t_
                return exp_t

            def phase_c(tt, exp_t):
                y = pcall[:, tt:tt + 1]
                t1 = smallp.tile([P, 1], f32, tag="sc")
                nc.vector.tensor_scalar(t1[:, :], y, 1.0, None, add)
                t2 = smallp.tile([P, 1], f32, tag="sc")
                nc.vector.tensor_tensor(t2[:, :], t1[:, :],
                                        S_all[:, tt:tt + 1], mult)
                t3 = smallp.tile([P, 1], f32, tag="sc")
                nc.vector.reciprocal(t3[:, :], t2[:, :])
                rs = smallp.tile([P, 1], f32, tag="sc")
                nc.vector.tensor_tensor(rs[:, :], t3[:, :], y, mult)
                for c0 in range(0, VSHP, 2048):
                    w = min(2048, VSHP - c0)
                    ost = outsp.tile([P, 2048], f32, tag="outst")
                    nc.vector.tensor_scalar(ost[:, :w], exp_t[:, c0:c0 + w],
                                            rs[:, :], None, mult)
                    nc.sync.dma_start(
                        out_p.ap()[tt * P:(tt + 1) * P, c0:c0 + w],
                        ost[:, :w])

            def copy_branch():
                for i in range(BSH):
                    pps = pssm.tile([P, 1], f32, tag="pc", bufs=1)
                    for k in range(NK):
                        nc.tensor.matmul(
                            pps[:, :], hcb_t[i][:, k * P:(k + 1) * P],
                            wcT[:, k:k + 1],
                            start=(k == 0), stop=(k == NK - 1))
                    ycb = smallp.tile([P, 1], f32, tag="sc")
                    nc.scalar.activation(ycb[:, :], pps[:, :], Exp,
                                         bias=bcNeg[:, :], scale=-1.0)
                    t1 = smallp.tile([P, 1], f32, tag="sc")
                    nc.vector.tensor_scalar(t1[:, :], ycb[:, :], 1.0, None, add)
                    pcb = smallp.tile([P, 1], f32, tag="sc")
                    nc.vector.reciprocal(pcb[:, :], t1[:, :])
                    cps = pscb.tile([P, C], f32, tag="cb", bufs=1)
                    for ks in range(NS):
                        nc.tensor.matmul(
                            cps[:, :], attnT[:, ks, i * P:(i + 1) * P],
                            src_t[i][:, ks, :],
                            start=(ks == 0), stop=(ks == NS - 1))
                    cstg = cbp.tile([P, C], f32, name=f"cst{i}")
                    nc.vector.tensor_scalar(cstg[:, :], cps[:, :], pcb[:, :],
                                            None, mult)
                    nc.sync.dma_start(out_c.ap()[i * P:(i + 1) * P, :],
                                      cstg[:, :])

            exp_tiles = {}
            groups = []
            t0 = 0
            for gsz in GROUPS:
                groups.append(list(range(t0, t0 + gsz)))
                t0 += gsz
            assert t0 == NT

            for g, grp in enumerate(groups):
                lsg = lsgp.tile([P, len(grp)], f32, tag="lsg")
                for j, tt in enumerate(grp):
                    exp_tiles[tt] = phase_a(tt, lsg, j)
                cc_in = dramp.tile([P, len(grp)], f32, tag="cc_in")
                cc_out = dramp.tile([P, len(grp)], f32, tag="cc_out")
                nc.sync.dma_start(cc_in[:, :], lsg[:, :])
                nc.gpsimd.collective_compute(
                    "AllReduce", add,
                    replica_groups=[list(range(NCORES))],
                    ins=[cc_in.opt()], outs=[cc_out.opt()],
                )
                nc.sync.dma_start(
                    S_all[:, grp[0]:grp[0] + len(grp)], cc_out[:, :])
                if g == len(groups) - 1:
                    # fill the final AllReduce's latency shadow
                    copy_branch()
                if g >= 1:
                    for tt in groups[g - 1]:
                        phase_c(tt, exp_tiles.pop(tt))
            for tt in groups[-1]:
                phase_c(tt, exp_tiles.pop(tt))

    nc.finalize()
    return nc


_CACHE = {}


def _get_nc():
    if "nc" not in _CACHE:
        _CACHE["nc"] = build_nc()
    return _CACHE["nc"]


def make_in_maps(hidden, attn, src_map, W, b, w_copy, b_copy, pad_idx):
    import ml_dtypes

    bF = ml_dtypes.bfloat16
    hidden = np.asarray(hidden, np.float32)
    attn = np.asarray(attn, np.float32)
    src_map = np.asarray(src_map, np.float32)
    W = np.asarray(W, np.float32)
    b = np.asarray(b, np.float32)
    w_copy = np.asarray(w_copy, np.float32)
    b_copy = np.asarray(b_copy, np.float32)
    pad = int(np.asarray(pad_idx))

    # hidden^T tiles: [tt, din, k*128+t]
    H3 = hidden.reshape(NT, P, NK, P).transpose(0, 3, 2, 1)
    H3 = np.ascontiguousarray(H3.reshape(NT, P, D).astype(bF))
    wcT = np.ascontiguousarray(w_copy.reshape(NK, P).T.astype(bF))
    bc = np.ascontiguousarray(b_copy.reshape(1, 1).astype(bF))

    in_maps = []
    for c in range(NCORES):
        lo, hi = c * VSH, (c + 1) * VSH
        Wp = np.zeros((VSHP, D), np.float32)
        Wp[:VSH] = W[lo:hi]
        wT = np.ascontiguousarray(
            Wp.reshape(VSHP, NK, P).transpose(1, 2, 0).astype(bF))
        bsl = np.full((VSHP,), -1e30, np.float32)
        bsl[:VSH] = b[lo:hi]
        if lo <= pad < hi:
            bsl[pad - lo] = -1e30
        b_rowA = np.ascontiguousarray(bsl.reshape(1, VSHP).astype(bF))
        a_sl = attn[c * BSH * T:(c + 1) * BSH * T]
        attnT = np.ascontiguousarray(
            a_sl.reshape(BSH * T, NS, P).transpose(1, 2, 0).astype(bF))
        s_sl = src_map[c * BSH:(c + 1) * BSH]
        srcm = np.ascontiguousarray(
            s_sl.reshape(BSH * NS, P, C).astype(bF))
        in_maps.append({
            "hT": H3,
            "wT": wT,
            "b_row": b_rowA,
            "w_copyT": wcT,
            "b_copy": bc,
            "attnT": attnT,
            "srcm": srcm,
            "hidden_cb": np.ascontiguousarray(H3[c * BSH:(c + 1) * BSH]),
        })
    return in_maps


def assemble(results):
    out_prob = np.concatenate(
        [np.asarray(r["out_prob"], np.float32)[:, :VSH] for r in results],
        axis=1)
    copy_prob = np.concatenate(
        [np.asarray(r["copy_prob"], np.float32) for r in results], axis=0)
    return np.concatenate([out_prob, copy_prob], axis=1)


FULL_CFG = dict(B=B, T=T, S=S, C=C, V=V, D=D)


def run(cfg, inputs, trace=False):
    """test.py interface: run(K.FULL_CFG, np_inputs, trace=True)."""
    nc = _get_nc()
    in_maps = make_in_maps(**inputs)
    res = run_bass_kernel_spmd(nc, in_maps, list(range(NCORES)), trace=trace)
    return assemble(res.results), res


def kernel(**inputs) -> np.ndarray:
    out, _ = run(FULL_CFG, inputs, trace=False)
    return out
